# revision 28
# baseline (speedup 1.0000x reference)
"""Trainium2 Bass kernel: Swin-style attention with relative position bias.

Problem: x[16,1024,256] -> qkv proj -> 8-head attention (N=1024, d=32) with
relative-position bias gathered from a 63x63 table -> out proj.

Sharding: data-parallel over batch, 2 batches per core, 8 cores, no
collectives.  Each core runs the full attention for its 2 batches.

Key device-side design (per core):
  * All matmuls bf16 (inputs cast on host/device), fp32 PSUM accumulate.
  * Scores computed TRANSPOSED: S[j', i] = q_i . k_{1023-j'} so that after
    exp the probability tile [j,i] is directly the stationary operand of the
    attn@V matmul (no transposes anywhere in the hot loop).  The key/value
    token axis is globally reversed (j' = 1023-j) which makes the bias
    window access pattern all-positive-stride (see below).
  * Softmax denominator comes free: V is augmented with a ones column, so
    the attn@V matmul produces [33,512] = [out^T | sumexp].
  * Relative bias: bias(j,i) = T[(yi-yj+31)*63 + (xi-xj+31)] is Toeplitz in
    the 2D grid.  exp(T) is precomputed ON DEVICE (tiny table), stored to a
    DRAM scratch padded to row-stride 64; then per (head, j-chunk) a
    "sliding window" DMA loads V[p,q] = expT[base(p)+q] (per-partition
    offset baked into the DMA access pattern).  The [128,512] multiplicative
    bias tile is then just a strided VIEW of the window: offset yi*64+xi.
    exp(S)*exp(bias) == exp(S+bias).
  * Reciprocal of sumexp via ScalarE exp(-ln(x)) (same activation table set
    as exp -> no table switches).
  * Final projection: out[i,o] = sum_c norm[c,i]*W[c,o] with b_out folded in
    as an extra K=1 matmul against a ones row.
"""

import os
import sys
from contextlib import ExitStack

import numpy as np

for _p in ("/opt/trn_rl_repo", os.path.expanduser("~/.axon_site/_ro/trn_rl_repo")):
    if os.path.isdir(_p) and _p not in sys.path:
        sys.path.insert(0, _p)
        break

import concourse.bass as bass
import concourse.tile as tile
from concourse import bacc, mybir
from concourse.bass_utils import run_bass_kernel_spmd

# Problem constants (hardcoded per spec).
B, N, C = 16, 1024, 256
H, D = 8, 32
IH = IW = 32
OUP = 256
SCALE = D ** -0.5
NCORES = 8
BPC = B // NCORES  # batches per core = 2
FP32 = mybir.dt.float32
BF16 = mybir.dt.bfloat16

_CACHE = {}


def _build_nc():
    nc = bacc.Bacc("TRN2", target_bir_lowering=False, debug=False)

    xT_ext = nc.dram_tensor("xT", [BPC, C, N], FP32, kind="ExternalInput")
    xTr_ext = nc.dram_tensor("xTr", [BPC, C, N], FP32, kind="ExternalInput")
    wqkv_ext = nc.dram_tensor("wqkv", [C, 3 * C], FP32, kind="ExternalInput")
    wout_ext = nc.dram_tensor("wout", [C, OUP], FP32, kind="ExternalInput")
    bout_ext = nc.dram_tensor("bout", [1, OUP], FP32, kind="ExternalInput")
    t2_ext = nc.dram_tensor("t2", [H, 4096], FP32, kind="ExternalInput")
    out_ext = nc.dram_tensor("out", [BPC, N, OUP], FP32, kind="ExternalOutput")

    expT2 = nc.dram_tensor("expT2", [H, 4096], BF16)  # device scratch

    Exp = mybir.ActivationFunctionType.Exp
    Ln = mybir.ActivationFunctionType.Ln
    Copy = mybir.ActivationFunctionType.Copy

    with tile.TileContext(nc) as tc:
        with ExitStack() as ctx:
            ent = ctx.enter_context
            # SBUF pools
            stage_pool = ent(tc.tile_pool(name="stage_f32", bufs=3))   # dma staging f32
            wq_pool = ent(tc.tile_pool(name="wq", bufs=2))             # wqkv bf16 [128,768]
            wo_pool = ent(tc.tile_pool(name="wo", bufs=5))             # wout bf16 + bout
            xtb_pool = ent(tc.tile_pool(name="xtb", bufs=4 * BPC))     # xT bf16 tiles
            qk_pool = ent(tc.tile_pool(name="qk", bufs=4 * BPC))       # qT/kTr bf16
            v_pool = ent(tc.tile_pool(name="vsb", bufs=BPC))           # v_sb [128, 2176]
            win_pool = ent(tc.tile_pool(name="win", bufs=3))           # bias windows
            sexp_pool = ent(tc.tile_pool(name="sexp", bufs=6))         # exp(S)
            outT_pool = ent(tc.tile_pool(name="outT", bufs=8 * BPC))   # [33,1024]/(b,h)
            sums_pool = ent(tc.tile_pool(name="sums", bufs=8))         # sumexp/ln/recip
            norm_pool = ent(tc.tile_pool(name="norm", bufs=2 * BPC))   # normalized outT
            fout_pool = ent(tc.tile_pool(name="fout", bufs=3))         # final f32 staging
            misc_pool = ent(tc.tile_pool(name="misc", bufs=1))         # preamble tiles
            # PSUM pools (8 banks total: 2x2-bank "s" slots + 4x1-bank "av")
            ps_s = ent(tc.tile_pool(name="ps_s", bufs=2, space="PSUM"))
            ps_av = ent(tc.tile_pool(name="ps_av", bufs=4, space="PSUM"))
            ps_m = ps_s
            # ---------------- Preamble: exp(bias table) -> DRAM scratch -----
            t2_sb = misc_pool.tile([H, 4096], FP32)
            nc.sync.dma_start(t2_sb[:], t2_ext[:])
            et2_sb = misc_pool.tile([H, 4096], BF16)
            nc.scalar.activation(et2_sb[:], t2_sb[:], Exp)
            nc.sync.dma_start(expT2[:], et2_sb[:])

            # ---------------- Weights to SBUF (bf16) -----------------------
            wqkv_sb = []
            for cc in range(2):
                st = stage_pool.tile([128, 3 * C], FP32, tag="wstage")
                nc.sync.dma_start(st[:], wqkv_ext[cc * 128:(cc + 1) * 128, :])
                wb = wq_pool.tile([128, 3 * C], BF16)
                nc.vector.tensor_copy(wb[:], st[:])
                wqkv_sb.append(wb)
            wout_sb = []
            for cc in range(2):
                st = stage_pool.tile([128, OUP], FP32, tag="wstage")
                nc.sync.dma_start(st[:], wout_ext[cc * 128:(cc + 1) * 128, :])
                wb = wo_pool.tile([128, OUP], BF16, tag="wout")
                nc.vector.tensor_copy(wb[:], st[:])
                wout_sb.append(wb)
            st = stage_pool.tile([1, OUP], FP32, tag="wstage")
            nc.sync.dma_start(st[:], bout_ext[:])
            bout_sb = wo_pool.tile([1, OUP], BF16, tag="wout")
            nc.vector.tensor_copy(bout_sb[:], st[:])
            ones_row = wo_pool.tile([1, 128], BF16, tag="wout")
            nc.gpsimd.memset(ones_row[:], 1.0)
            # ones column block: K=1 lhsT slices at any 32-aligned partition,
            # used to broadcast recip rows across partitions via matmul.
            ones_col = wo_pool.tile([128, 32], BF16, tag="wout")
            nc.gpsimd.memset(ones_col[:], 1.0)

            # ---------------- x^T (and reversed) to SBUF bf16 ---------------
            xTb = [[None, None] for _ in range(BPC)]
            xTrb = [[None, None] for _ in range(BPC)]
            for b in range(BPC):
                for cc in range(2):
                    for (ext, dst) in ((xT_ext, xTb), (xTr_ext, xTrb)):
                        st = stage_pool.tile([128, N], FP32, tag="xstage")
                        nc.sync.dma_start(
                            st[:], ext[b, cc * 128:(cc + 1) * 128, :])
                        xb = xtb_pool.tile([128, N], BF16)
                        nc.vector.tensor_copy(xb[:], st[:])
                        dst[b][cc] = xb

            # ---------------- QKV projections ------------------------------
            # q^T / kTr^T: out[col_chunk(128), i(1024)]; cols 0-255 = q (rhs
            # xT, normal token order), cols 256-511 = k (rhs xTr, reversed).
            qT_sb = [[None, None] for _ in range(BPC)]   # [b][chunk] chunks: q cols
            kTr_sb = [[None, None] for _ in range(BPC)]
            v_sb = [None] * BPC
            for b in range(BPC):
                for m in range(4):
                    dst_list, dst_idx = (qT_sb, m) if m < 2 else (kTr_sb, m - 2)
                    rhs_src = xTb if m < 2 else xTrb
                    dst = qk_pool.tile([128, N], BF16)
                    for half in range(2):
                        ps = ps_s.tile([128, 512], FP32, tag="s")
                        for cc in range(2):
                            nc.tensor.matmul(
                                ps[:],
                                wqkv_sb[cc][:, m * 128:(m + 1) * 128],
                                rhs_src[b][cc][:, half * 512:(half + 1) * 512],
                                start=(cc == 0), stop=(cc == 1),
                            )
                        nc.vector.tensor_copy(
                            dst[:, half * 512:(half + 1) * 512], ps[:])
                    dst_list[b][dst_idx] = dst

                # v: out[token'(128-chunk), vcol(256)], token order reversed
                # (lhsT = xTr chunk).  Stored strided: per (chunk jc, head h)
                # 34 cols = [v(32) | 1.0 | pad]; ones from the initial memset.
                vb = v_pool.tile([128, 8 * 272], BF16)
                nc.gpsimd.memset(vb[:], 1.0)
                for tc_ in range(8):
                    ps = ps_m.tile([128, OUP], FP32, tag="s")
                    for cc in range(2):
                        nc.tensor.matmul(
                            ps[:],
                            xTrb[b][cc][:, tc_ * 128:(tc_ + 1) * 128],
                            wqkv_sb[cc][:, 512:768],
                            start=(cc == 0), stop=(cc == 1),
                        )
                    dst_ap = vb[:].rearrange(
                        "p (j h d) -> p j h d", h=8, d=34
                    )[:, tc_:tc_ + 1, :, 0:32].squeeze(1)
                    nc.vector.tensor_copy(
                        dst_ap, ps[:].rearrange("p (h d) -> p h d", d=32))
                v_sb[b] = vb

            # ---------------- Attention ------------------------------------
            # S'[j',i] tile per (h, jc, b, half); windows shared across b.
            outT_sb = {}
            # sumexp rows packed at partitions {0,32,64,96} of 2 tiles per b
            # (engine APs need 32-aligned start partitions); padding=1.0 so
            # ln/exp over the full tile stays finite.
            sumexp_sb = []
            for _b in range(BPC):
                pair = []
                for _t in range(2):
                    se_tile = sums_pool.tile([128, N], BF16, tag="sums",
                                             name=f"se{_b}_{_t}")
                    nc.gpsimd.memset(se_tile[:], 1.0)
                    pair.append(se_tile)
                sumexp_sb.append(pair)
            dma_engines = [nc.sync, nc.gpsimd, nc.scalar]
            for h in range(H):
                hc, hr = h // 4, (h % 4) * 32
                av = {}
                for jc in range(8):
                    win = win_pool.tile([128, 2048], BF16)
                    src = bass.AP(
                        tensor=expT2.ap().tensor,
                        offset=h * 4096 + jc * 256,
                        ap=[[64, 4], [1, 32], [1, 2048]],
                    )
                    dma_engines[(h * 8 + jc) % 3].dma_start(win[:], src)
                    win3 = win[:].rearrange("p (y x) -> p y x", x=64)
                    for b in range(BPC):
                        ps = ps_s.tile([128, N], FP32, tag="s")
                        for half in range(2):
                            nc.tensor.matmul(
                                ps[:, half * 512:(half + 1) * 512],
                                kTr_sb[b][hc][hr:hr + 32, jc * 128:(jc + 1) * 128],
                                qT_sb[b][hc][hr:hr + 32, half * 512:(half + 1) * 512],
                                start=True, stop=True,
                                tile_position=(hr, 0),
                            )
                        sraw = sexp_pool.tile([128, N], BF16, tag="sraw")
                        nc.scalar.activation(sraw[:], ps[:], Exp, scale=SCALE)
                        sexp = sexp_pool.tile([128, N], BF16, tag="sexp")
                        nc.vector.tensor_mul(
                            sexp[:].rearrange("p (a x) -> p a x", x=32),
                            sraw[:].rearrange("p (a x) -> p a x", x=32),
                            win3[:, 0:32, 0:32],
                        )
                        for half in range(2):
                            if jc == 0:
                                av[(b, half)] = ps_av.tile(
                                    [33, 512], FP32, tag="av",
                                    name=f"av{h}_{b}_{half}")
                            nc.tensor.matmul(
                                av[(b, half)][:],
                                v_sb[b][:, jc * 272 + h * 34: jc * 272 + h * 34 + 33],
                                sexp[:, half * 512:(half + 1) * 512],
                                start=(jc == 0), stop=(jc == 7),
                            )
                # evict: rows 0-31 -> outT (bf16), row 32 -> sumexp row h;
                # half 0 on ACT, half 1 on DVE (engine balance)
                for b in range(BPC):
                    ot = outT_pool.tile([33, N], BF16)
                    nc.scalar.activation(ot[:, 0:512], av[(b, 0)][:], Copy)
                    nc.vector.tensor_copy(ot[:, 512:1024], av[(b, 1)][:])
                    nc.vector.tensor_copy(
                        sumexp_sb[b][h // 4][(h % 4) * 32:(h % 4) * 32 + 1, :],
                        ot[32:33, :])
                    outT_sb[(b, h)] = ot

            # ---------------- Normalize + final projection -----------------
            for b in range(BPC):
                recips = []
                for t in range(2):
                    lns = sums_pool.tile([128, N], BF16, tag="sums",
                                         name=f"lns{b}_{t}")
                    nc.scalar.activation(lns[:], sumexp_sb[b][t][:], Ln)
                    rec = sums_pool.tile([128, N], BF16, tag="sums",
                                         name=f"rec{b}_{t}")
                    nc.scalar.activation(rec[:], lns[:], Exp, scale=-1.0)
                    recips.append(rec)

                normt = [norm_pool.tile([128, N], BF16, tag="normt",
                                        name=f"normt{b}_{i}") for i in range(2)]
                for h in range(H):
                    hc, hr = h // 4, (h % 4) * 32
                    # broadcast recip row across 32 partitions via K=1 matmul
                    bc = ps_s.tile([32, N], FP32, tag="s",
                                   name=f"bc{b}_{h}")
                    for half in range(2):
                        nc.tensor.matmul(
                            bc[:, half * 512:(half + 1) * 512],
                            ones_col[hr:hr + 1, :],
                            recips[h // 4][hr:hr + 1,
                                           half * 512:(half + 1) * 512],
                            start=True, stop=True,
                            tile_position=(hr, 0),
                        )
                    nc.vector.tensor_mul(
                        normt[hc][hr:hr + 32, :],
                        outT_sb[(b, h)][0:32, :],
                        bc[:],
                    )
                for ic in range(8):
                    ps = ps_m.tile([128, OUP], FP32, tag="s",
                                   name=f"fps{b}_{ic}")
                    nc.tensor.matmul(ps[:], normt[0][:, ic * 128:(ic + 1) * 128],
                                     wout_sb[0][:], start=True, stop=False)
                    nc.tensor.matmul(ps[:], normt[1][:, ic * 128:(ic + 1) * 128],
                                     wout_sb[1][:], start=False, stop=False)
                    nc.tensor.matmul(ps[:], ones_row[:], bout_sb[:],
                                     start=False, stop=True)
                    fo = fout_pool.tile([128, OUP], FP32)
                    nc.vector.tensor_copy(fo[:], ps[:])
                    nc.sync.dma_start(
                        out_ext[b, ic * 128:(ic + 1) * 128, :], fo[:])

    nc.compile()
    return nc


def _host_prep(x, W_qkv, W_out, b_out, bias_table):
    """Pure layout prep (shard / transpose / pad) -- no arithmetic."""
    x = np.asarray(x, dtype=np.float32)
    # T2[h, dy*64+dx] = bias_table[dy*63+dx, h]; rows padded 63->64, tail 0.
    t2 = np.zeros((H, 4096), dtype=np.float32)
    bt = np.asarray(bias_table, dtype=np.float32)  # [3969, 8]
    t2_rows = bt.T.reshape(H, 63, 63)              # [h, dy, dx]
    t2.reshape(H, 64, 64)[:, :63, :63] = t2_rows
    in_maps = []
    for c in range(NCORES):
        xs = x[c * BPC:(c + 1) * BPC]                        # [2, N, C]
        xT = np.ascontiguousarray(xs.transpose(0, 2, 1))     # [2, C, N]
        xTr = np.ascontiguousarray(xT[:, :, ::-1])
        in_maps.append({
            "xT": xT,
            "xTr": xTr,
            "wqkv": np.ascontiguousarray(W_qkv, dtype=np.float32),
            "wout": np.ascontiguousarray(W_out, dtype=np.float32),
            "bout": np.ascontiguousarray(
                np.asarray(b_out, dtype=np.float32).reshape(1, OUP)),
            "t2": t2,
        })
    return in_maps


def kernel(x, W_qkv, W_out, b_out, bias_table, rel_index=None, **_unused):
    if "nc" not in _CACHE:
        _CACHE["nc"] = _build_nc()
    nc = _CACHE["nc"]
    in_maps = _host_prep(x, W_qkv, W_out, b_out, bias_table)
    res = run_bass_kernel_spmd(nc, in_maps, core_ids=list(range(NCORES)))
    out = np.empty((B, N, OUP), dtype=np.float32)
    for c in range(NCORES):
        out[c * BPC:(c + 1) * BPC] = res.results[c]["out"]
    return out


if __name__ == "__main__":
    rng = np.random.default_rng(0)
    xs = rng.standard_normal((B, N, C), dtype=np.float32)
    wq = rng.standard_normal((C, 3 * C), dtype=np.float32) * 0.02
    wo = rng.standard_normal((C, OUP), dtype=np.float32) * 0.02
    bo = np.zeros((OUP,), dtype=np.float32)
    bt = rng.standard_normal(((2 * IH - 1) * (2 * IW - 1), H),
                             dtype=np.float32) * 0.02
    o = kernel(xs, wq, wo, bo, bt)
    print("kernel output", o.shape, o.dtype, float(np.abs(o).mean()))


# revision 31
# speedup vs baseline: 1.4152x; 1.4152x over previous
"""Trainium2 Bass kernel: Swin-style attention with relative position bias.

Problem: x[16,1024,256] -> qkv proj -> 8-head attention (N=1024, d=32) with
relative-position bias gathered from a 63x63 table -> out proj.

Sharding: data-parallel over batch, 2 batches per core, 8 cores, no
collectives.  Each core runs the full attention for its 2 batches.

Key device-side design (per core):
  * All matmuls bf16 (inputs cast on host/device), fp32 PSUM accumulate.
  * Scores computed TRANSPOSED: S[j', i] = q_i . k_{1023-j'} so that after
    exp the probability tile [j,i] is directly the stationary operand of the
    attn@V matmul (no transposes anywhere in the hot loop).  The key/value
    token axis is globally reversed (j' = 1023-j) which makes the bias
    window access pattern all-positive-stride (see below).
  * Softmax denominator comes free: V is augmented with a ones column, so
    the attn@V matmul produces [33,512] = [out^T | sumexp].
  * Relative bias: bias(j,i) = T[(yi-yj+31)*63 + (xi-xj+31)] is Toeplitz in
    the 2D grid.  exp(T) is precomputed ON DEVICE (tiny table), stored to a
    DRAM scratch padded to row-stride 64; then per (head, j-chunk) a
    "sliding window" DMA loads V[p,q] = expT[base(p)+q] (per-partition
    offset baked into the DMA access pattern).  The [128,512] multiplicative
    bias tile is then just a strided VIEW of the window: offset yi*64+xi.
    exp(S)*exp(bias) == exp(S+bias).
  * Reciprocal of sumexp via ScalarE exp(-ln(x)) (same activation table set
    as exp -> no table switches).
  * Final projection: out[i,o] = sum_c norm[c,i]*W[c,o] with b_out folded in
    as an extra K=1 matmul against a ones row.
"""

import os
import sys
from contextlib import ExitStack

import numpy as np

for _p in ("/opt/trn_rl_repo", os.path.expanduser("~/.axon_site/_ro/trn_rl_repo")):
    if os.path.isdir(_p) and _p not in sys.path:
        sys.path.insert(0, _p)
        break

import concourse.bass as bass
import concourse.tile as tile
from concourse import bacc, mybir
from concourse.bass_utils import run_bass_kernel_spmd

# Problem constants (hardcoded per spec).
B, N, C = 16, 1024, 256
H, D = 8, 32
IH = IW = 32
OUP = 256
SCALE = D ** -0.5
NCORES = 8
BPC = B // NCORES  # batches per core = 2
FP32 = mybir.dt.float32
BF16 = mybir.dt.bfloat16

_CACHE = {}


def _build_nc():
    nc = bacc.Bacc("TRN2", target_bir_lowering=False, debug=False)

    xT_ext = nc.dram_tensor("xT", [BPC, C, N], FP32, kind="ExternalInput")
    xTr_ext = nc.dram_tensor("xTr", [BPC, C, N], FP32, kind="ExternalInput")
    wqkv_ext = nc.dram_tensor("wqkv", [C, 3 * C], FP32, kind="ExternalInput")
    wout_ext = nc.dram_tensor("wout", [C, OUP], FP32, kind="ExternalInput")
    bout_ext = nc.dram_tensor("bout", [1, OUP], FP32, kind="ExternalInput")
    t2_ext = nc.dram_tensor("t2", [H, 4096], FP32, kind="ExternalInput")
    out_ext = nc.dram_tensor("out", [BPC, N, OUP], FP32, kind="ExternalOutput")

    expT2 = nc.dram_tensor("expT2", [H, 4096], BF16)  # device scratch

    Exp = mybir.ActivationFunctionType.Exp
    Ln = mybir.ActivationFunctionType.Ln
    Copy = mybir.ActivationFunctionType.Copy

    with tile.TileContext(nc) as tc:
        with ExitStack() as ctx:
            ent = ctx.enter_context
            # SBUF pools
            stage_pool = ent(tc.tile_pool(name="stage_f32", bufs=3))   # dma staging f32
            wq_pool = ent(tc.tile_pool(name="wq", bufs=2))             # wqkv bf16 [128,768]
            wo_pool = ent(tc.tile_pool(name="wo", bufs=5))             # wout bf16 + bout
            xtb_pool = ent(tc.tile_pool(name="xtb", bufs=4 * BPC))     # xT bf16 tiles
            qk_pool = ent(tc.tile_pool(name="qk", bufs=4 * BPC))       # qT/kTr bf16
            v_pool = ent(tc.tile_pool(name="vsb", bufs=BPC))           # v_sb [128, 2176]
            win_pool = ent(tc.tile_pool(name="win", bufs=3))           # bias windows
            sexp_pool = ent(tc.tile_pool(name="sexp", bufs=6))         # exp(S)
            outT_pool = ent(tc.tile_pool(name="outT", bufs=8 * BPC))   # [33,1024]/(b,h)
            sums_pool = ent(tc.tile_pool(name="sums", bufs=8))         # sumexp/ln/recip
            norm_pool = ent(tc.tile_pool(name="norm", bufs=2 * BPC))   # normalized outT
            fout_pool = ent(tc.tile_pool(name="fout", bufs=3))         # final f32 staging
            misc_pool = ent(tc.tile_pool(name="misc", bufs=1))         # preamble tiles
            # PSUM pools (8 banks total: 2x2-bank "s" slots + 4x1-bank "av")
            ps_s = ent(tc.tile_pool(name="ps_s", bufs=2, space="PSUM"))
            ps_av = ent(tc.tile_pool(name="ps_av", bufs=4, space="PSUM"))
            ps_m = ps_s
            # ---------------- Preamble: exp(bias table) -> DRAM scratch -----
            t2_sb = misc_pool.tile([H, 4096], FP32)
            nc.sync.dma_start(t2_sb[:], t2_ext[:])
            et2_sb = misc_pool.tile([H, 4096], BF16)
            nc.scalar.activation(et2_sb[:], t2_sb[:], Exp)
            nc.sync.dma_start(expT2[:], et2_sb[:])

            # ---------------- Weights to SBUF (bf16) -----------------------
            wqkv_sb = []
            for cc in range(2):
                st = stage_pool.tile([128, 3 * C], FP32, tag="wstage")
                nc.sync.dma_start(st[:], wqkv_ext[cc * 128:(cc + 1) * 128, :])
                wb = wq_pool.tile([128, 3 * C], BF16)
                nc.vector.tensor_copy(wb[:], st[:])
                wqkv_sb.append(wb)
            wout_sb = []
            for cc in range(2):
                st = stage_pool.tile([128, OUP], FP32, tag="wstage")
                nc.sync.dma_start(st[:], wout_ext[cc * 128:(cc + 1) * 128, :])
                wb = wo_pool.tile([128, OUP], BF16, tag="wout")
                nc.vector.tensor_copy(wb[:], st[:])
                wout_sb.append(wb)
            st = stage_pool.tile([1, OUP], FP32, tag="wstage")
            nc.sync.dma_start(st[:], bout_ext[:])
            bout_sb = wo_pool.tile([1, OUP], BF16, tag="wout")
            nc.vector.tensor_copy(bout_sb[:], st[:])
            ones_row = wo_pool.tile([1, 128], BF16, tag="wout")
            nc.gpsimd.memset(ones_row[:], 1.0)
            # ones column block: K=1 lhsT slices at any 32-aligned partition,
            # used to broadcast recip rows across partitions via matmul.
            ones_col = wo_pool.tile([128, 32], BF16, tag="wout")
            nc.gpsimd.memset(ones_col[:], 1.0)

            # ---------------- x^T (and reversed) to SBUF bf16 ---------------
            xTb = [[None, None] for _ in range(BPC)]
            xTrb = [[None, None] for _ in range(BPC)]
            for b in range(BPC):
                for cc in range(2):
                    for (ext, dst) in ((xT_ext, xTb), (xTr_ext, xTrb)):
                        st = stage_pool.tile([128, N], FP32, tag="xstage")
                        nc.sync.dma_start(
                            st[:], ext[b, cc * 128:(cc + 1) * 128, :])
                        xb = xtb_pool.tile([128, N], BF16)
                        nc.vector.tensor_copy(xb[:], st[:])
                        dst[b][cc] = xb

            # ---------------- QKV projections ------------------------------
            # q^T / kTr^T: out[col_chunk(128), i(1024)]; cols 0-255 = q (rhs
            # xT, normal token order), cols 256-511 = k (rhs xTr, reversed).
            qT_sb = [[None, None] for _ in range(BPC)]   # [b][chunk] chunks: q cols
            kTr_sb = [[None, None] for _ in range(BPC)]
            v_sb = [None] * BPC
            for b in range(BPC):
                for m in range(4):
                    dst_list, dst_idx = (qT_sb, m) if m < 2 else (kTr_sb, m - 2)
                    rhs_src = xTb if m < 2 else xTrb
                    dst = qk_pool.tile([128, N], BF16)
                    for half in range(2):
                        ps = ps_s.tile([128, 512], FP32, tag="s")
                        for cc in range(2):
                            nc.tensor.matmul(
                                ps[:],
                                wqkv_sb[cc][:, m * 128:(m + 1) * 128],
                                rhs_src[b][cc][:, half * 512:(half + 1) * 512],
                                start=(cc == 0), stop=(cc == 1),
                            )
                        nc.vector.tensor_copy(
                            dst[:, half * 512:(half + 1) * 512], ps[:])
                    dst_list[b][dst_idx] = dst

                # v: out[token'(128-chunk), vcol(256)], token order reversed
                # (lhsT = xTr chunk).  Stored strided: per (chunk jc, head h)
                # 34 cols = [v(32) | 1.0 | pad]; ones from the initial memset.
                vb = v_pool.tile([128, 8 * 272], BF16)
                nc.gpsimd.memset(vb[:], 1.0)
                for tc_ in range(8):
                    ps = ps_m.tile([128, OUP], FP32, tag="s")
                    for cc in range(2):
                        nc.tensor.matmul(
                            ps[:],
                            xTrb[b][cc][:, tc_ * 128:(tc_ + 1) * 128],
                            wqkv_sb[cc][:, 512:768],
                            start=(cc == 0), stop=(cc == 1),
                        )
                    dst_ap = vb[:].rearrange(
                        "p (j h d) -> p j h d", h=8, d=34
                    )[:, tc_:tc_ + 1, :, 0:32].squeeze(1)
                    nc.vector.tensor_copy(
                        dst_ap, ps[:].rearrange("p (h d) -> p h d", d=32))
                v_sb[b] = vb

            # ---------------- Attention ------------------------------------
            # S'[j',i] tile per (h, jc, b, half); windows shared across b.
            outT_sb = {}
            # sumexp rows packed at partitions {0,32,64,96} of 2 tiles per b
            # (engine APs need 32-aligned start partitions); padding=1.0 so
            # ln/exp over the full tile stays finite.
            sumexp_sb = []
            for _b in range(BPC):
                pair = []
                for _t in range(2):
                    se_tile = sums_pool.tile([128, N], BF16, tag="sums",
                                             name=f"se{_b}_{_t}")
                    nc.gpsimd.memset(se_tile[:], 1.0)
                    pair.append(se_tile)
                sumexp_sb.append(pair)
            dma_engines = [nc.sync, nc.scalar]
            for h in range(H):
                hc, hr = h // 4, (h % 4) * 32
                # One bias window per head: W2[p, q] = expT2[h, q + shift_p]
                # with shift_p = (p//32)*64 + p%32 (the within-chunk offset);
                # the per-j-chunk base jc*256 becomes a view offset.
                win = win_pool.tile([128, 3840], BF16)
                src = bass.AP(
                    tensor=expT2.ap().tensor,
                    offset=h * 4096,
                    ap=[[64, 4], [1, 32], [1, 3840]],
                )
                dma_engines[h % 2].dma_start(win[:], src)
                win3 = win[:].rearrange("p (y q) -> p y q", q=64)
                av = {}
                for jc in range(8):
                    for b in range(BPC):
                        ps = ps_s.tile([128, N], FP32, tag="s")
                        for half in range(2):
                            nc.tensor.matmul(
                                ps[:, half * 512:(half + 1) * 512],
                                kTr_sb[b][hc][hr:hr + 32, jc * 128:(jc + 1) * 128],
                                qT_sb[b][hc][hr:hr + 32, half * 512:(half + 1) * 512],
                                start=True, stop=True,
                                tile_position=(hr, 0),
                            )
                        sraw = sexp_pool.tile([128, N], BF16, tag="sraw")
                        nc.scalar.activation(sraw[:], ps[:], Exp, scale=SCALE)
                        sexp = sexp_pool.tile([128, N], BF16, tag="sexp")
                        nc.vector.tensor_mul(
                            sexp[:].rearrange("p (a x) -> p a x", x=32),
                            sraw[:].rearrange("p (a x) -> p a x", x=32),
                            win3[:, jc * 4:jc * 4 + 32, 0:32],
                        )
                        for half in range(2):
                            if jc == 0:
                                av[(b, half)] = ps_av.tile(
                                    [33, 512], FP32, tag="av",
                                    name=f"av{h}_{b}_{half}")
                            nc.tensor.matmul(
                                av[(b, half)][:],
                                v_sb[b][:, jc * 272 + h * 34: jc * 272 + h * 34 + 33],
                                sexp[:, half * 512:(half + 1) * 512],
                                start=(jc == 0), stop=(jc == 7),
                            )
                # evict: rows 0-31 -> outT (bf16), row 32 -> sumexp row h;
                # half 0 on ACT, half 1 on DVE (engine balance)
                for b in range(BPC):
                    ot = outT_pool.tile([33, N], BF16)
                    nc.scalar.activation(ot[:, 0:512], av[(b, 0)][:], Copy)
                    nc.vector.tensor_copy(ot[:, 512:1024], av[(b, 1)][:])
                    nc.vector.tensor_copy(
                        sumexp_sb[b][h // 4][(h % 4) * 32:(h % 4) * 32 + 1, :],
                        ot[32:33, :])
                    outT_sb[(b, h)] = ot

            # ---------------- Normalize + final projection -----------------
            for b in range(BPC):
                recips = []
                for t in range(2):
                    lns = sums_pool.tile([128, N], BF16, tag="sums",
                                         name=f"lns{b}_{t}")
                    nc.scalar.activation(lns[:], sumexp_sb[b][t][:], Ln)
                    rec = sums_pool.tile([128, N], BF16, tag="sums",
                                         name=f"rec{b}_{t}")
                    nc.scalar.activation(rec[:], lns[:], Exp, scale=-1.0)
                    recips.append(rec)

                normt = [norm_pool.tile([128, N], BF16, tag="normt",
                                        name=f"normt{b}_{i}") for i in range(2)]
                for h in range(H):
                    hc, hr = h // 4, (h % 4) * 32
                    # broadcast recip row across 32 partitions via K=1 matmul
                    bc = ps_s.tile([32, N], FP32, tag="s",
                                   name=f"bc{b}_{h}")
                    for half in range(2):
                        nc.tensor.matmul(
                            bc[:, half * 512:(half + 1) * 512],
                            ones_col[hr:hr + 1, :],
                            recips[h // 4][hr:hr + 1,
                                           half * 512:(half + 1) * 512],
                            start=True, stop=True,
                            tile_position=(hr, 0),
                        )
                    nc.vector.tensor_mul(
                        normt[hc][hr:hr + 32, :],
                        outT_sb[(b, h)][0:32, :],
                        bc[:],
                    )
                for ic in range(8):
                    ps = ps_m.tile([128, OUP], FP32, tag="s",
                                   name=f"fps{b}_{ic}")
                    nc.tensor.matmul(ps[:], normt[0][:, ic * 128:(ic + 1) * 128],
                                     wout_sb[0][:], start=True, stop=False)
                    nc.tensor.matmul(ps[:], normt[1][:, ic * 128:(ic + 1) * 128],
                                     wout_sb[1][:], start=False, stop=False)
                    nc.tensor.matmul(ps[:], ones_row[:], bout_sb[:],
                                     start=False, stop=True)
                    fo = fout_pool.tile([128, OUP], FP32)
                    nc.vector.tensor_copy(fo[:], ps[:])
                    nc.scalar.dma_start(
                        out_ext[b, ic * 128:(ic + 1) * 128, :], fo[:])

    nc.compile()
    return nc


def _host_prep(x, W_qkv, W_out, b_out, bias_table):
    """Pure layout prep (shard / transpose / pad) -- no arithmetic."""
    x = np.asarray(x, dtype=np.float32)
    # T2[h, dy*64+dx] = bias_table[dy*63+dx, h]; rows padded 63->64, tail 0.
    t2 = np.zeros((H, 4096), dtype=np.float32)
    bt = np.asarray(bias_table, dtype=np.float32)  # [3969, 8]
    t2_rows = bt.T.reshape(H, 63, 63)              # [h, dy, dx]
    t2.reshape(H, 64, 64)[:, :63, :63] = t2_rows
    in_maps = []
    for c in range(NCORES):
        xs = x[c * BPC:(c + 1) * BPC]                        # [2, N, C]
        xT = np.ascontiguousarray(xs.transpose(0, 2, 1))     # [2, C, N]
        xTr = np.ascontiguousarray(xT[:, :, ::-1])
        in_maps.append({
            "xT": xT,
            "xTr": xTr,
            "wqkv": np.ascontiguousarray(W_qkv, dtype=np.float32),
            "wout": np.ascontiguousarray(W_out, dtype=np.float32),
            "bout": np.ascontiguousarray(
                np.asarray(b_out, dtype=np.float32).reshape(1, OUP)),
            "t2": t2,
        })
    return in_maps


def kernel(x, W_qkv, W_out, b_out, bias_table, rel_index=None, **_unused):
    if "nc" not in _CACHE:
        _CACHE["nc"] = _build_nc()
    nc = _CACHE["nc"]
    in_maps = _host_prep(x, W_qkv, W_out, b_out, bias_table)
    res = run_bass_kernel_spmd(nc, in_maps, core_ids=list(range(NCORES)))
    out = np.empty((B, N, OUP), dtype=np.float32)
    for c in range(NCORES):
        out[c * BPC:(c + 1) * BPC] = res.results[c]["out"]
    return out


if __name__ == "__main__":
    rng = np.random.default_rng(0)
    xs = rng.standard_normal((B, N, C), dtype=np.float32)
    wq = rng.standard_normal((C, 3 * C), dtype=np.float32) * 0.02
    wo = rng.standard_normal((C, OUP), dtype=np.float32) * 0.02
    bo = np.zeros((OUP,), dtype=np.float32)
    bt = rng.standard_normal(((2 * IH - 1) * (2 * IW - 1), H),
                             dtype=np.float32) * 0.02
    o = kernel(xs, wq, wo, bo, bt)
    print("kernel output", o.shape, o.dtype, float(np.abs(o).mean()))
